# revision 7
# baseline (speedup 1.0000x reference)
"""Trainium2 Bass kernel for AtomGraphGINE message passing (8 NeuronCores).

Distribution: nodes+edges sharded by graph (batch is sorted, shards are graph
aligned). Weights replicated. Per layer: AllGather of node states h into a
replicated DRAM table, per-edge rows gathered with indirect DMA, message
relu(h[src]+e) formed in PSUM, scatter-add to destination nodes via one-hot
matmuls accumulated per 128-node window, dense node update with BatchNorm
(global stats via small AllReduce, padding corrected analytically), final
per-graph mean pooling via exclusive prefix sums and two indirect gathers.
"""

import sys
import types

import numpy as np

M = 8          # NeuronCores
G_DEFAULT = 4096
BN_EPS = 1e-5
SBW = 4        # windows per gather superbatch (SBUF staging granularity)


def _register_ntff_hook():
    if "antenv.axon_hooks" in sys.modules:
        return
    try:
        import antenv
    except ImportError:
        return
    mod = types.ModuleType("antenv.axon_hooks")
    mod._hook = None

    def set_axon_ntff_profile_hook(h):
        mod._hook = h

    def get_axon_ntff_profile_hook():
        return mod._hook

    mod.set_axon_ntff_profile_hook = set_axon_ntff_profile_hook
    mod.get_axon_ntff_profile_hook = get_axon_ntff_profile_hook
    sys.modules["antenv.axon_hooks"] = mod
    antenv.axon_hooks = mod
    try:
        from trn_agent_boot.trn_boot import _ntff_profile_via_ctypes
        set_axon_ntff_profile_hook(_ntff_profile_via_ctypes("/opt/axon/libaxon_pjrt.so"))
    except Exception:
        pass


def _round_up(x, m):
    return int((x + m - 1) // m) * m


def preprocess(x, edge_attr, embW, embB, bondW, bondB, W1, b1, g1, be1,
               W2, b2, gout, bout, edge_index, batch, G):
    x = np.asarray(x, np.float32)
    edge_attr = np.asarray(edge_attr, np.float32)
    src = np.asarray(edge_index[0], np.int64)
    dst = np.asarray(edge_index[1], np.int64)
    batch = np.asarray(batch, np.int64)
    N, ATOM = x.shape
    E = src.shape[0]
    BOND = edge_attr.shape[1]
    D = np.asarray(embW).shape[1]
    L = np.asarray(bondW).shape[0]

    # ---- graph-aligned node partition over M cores ----
    gstarts = np.searchsorted(batch, np.arange(G + 1))  # node start of each graph
    ideal = (np.arange(M + 1) * N) // M
    gsel = np.searchsorted(gstarts, ideal)
    gsel = np.clip(gsel, 0, G)
    gsel[0], gsel[M] = 0, G
    for c in range(1, M):  # snap to nearest boundary, keep monotone
        lo = max(gsel[c] - 1, gsel[c - 1] + 1)
        hi = min(gsel[c] + 1, gsel[c + 1] - 1) if c < M else gsel[c]
        best, bestd = gsel[c], abs(int(gstarts[gsel[c]]) - int(ideal[c]))
        for g in range(lo, hi + 1):
            d = abs(int(gstarts[g]) - int(ideal[c]))
            if d < bestd:
                best, bestd = g, d
        gsel[c] = best
    graph_start = gsel.astype(np.int64)
    node_start = gstarts[graph_start].astype(np.int64)
    n_real = np.diff(node_start)
    g_real = np.diff(graph_start)
    assert (n_real > 0).all()

    Npad = _round_up(int(n_real.max()) + 1, 512)
    NW = Npad // 128
    NT = Npad // 512
    NSB = NW // SBW

    # ---- edge partition by dst owner; window = dst_local // 128 ----
    owner = np.searchsorted(node_start, dst, side="right") - 1
    dst_local = dst - node_start[owner]
    w_dst = dst_local // 128
    srco = np.searchsorted(node_start, src, side="right") - 1
    grow = srco * Npad + (src - node_start[srco])  # row in the AG'd table

    key = owner * NW + w_dst
    cnt = np.bincount(key, minlength=M * NW)
    CPW = max(1, int(-(-int(cnt.max()) // 128)))
    SLOTW = CPW * 128
    EC = NW * CPW            # index columns per core
    ES = NW * SLOTW          # edge slots per core

    order = np.lexsort((grow, key))
    key_s = key[order]
    grow_s = grow[order]
    dstl_s = dst_local[order]
    ea_s = edge_attr[order]
    # position within each (owner, window) group
    firsts = np.searchsorted(key_s, np.arange(M * NW))
    within = np.arange(E) - firsts[key_s]
    slot = (key_s % NW) * SLOTW + within          # slot within the core
    core_e = key_s // NW
    p = slot % 128
    col = slot // 128

    gidx = np.zeros((M, 128, EC), np.int32)
    dstin = np.full((M, 128, EC), -1.0, np.float32)
    eattrT = np.zeros((M, BOND + 1, ES), np.float32)
    gidx[core_e, p, col] = grow_s.astype(np.int32)
    dstin[core_e, p, col] = (dstl_s % 128).astype(np.float32)
    eattrT[core_e[:, None], np.arange(BOND)[None, :], slot[:, None]] = ea_s
    eattrT[core_e, BOND, slot] = 1.0

    # ---- per-core transposed, padded node features ----
    xT = np.zeros((M, ATOM + 1, Npad), np.float32)
    for c in range(M):
        xs = x[node_start[c]:node_start[c + 1]]
        xT[c, :ATOM, :n_real[c]] = xs.T
        xT[c, ATOM, :n_real[c]] = 1.0

    kc = np.zeros((M, 128, 1), np.float32)
    for c in range(M):
        kc[c, :, 0] = float(Npad - n_real[c])

    # ---- pooling tables (exclusive prefix rows) ----
    Gpad = _round_up(max(int(g_real.max()), 1), 128)
    NPW = Gpad // 128
    plo = np.zeros((M, 128, NPW), np.int32)
    phi = np.zeros((M, 128, NPW), np.int32)
    pscale = np.zeros((M, 128, NPW), np.float32)
    for c in range(M):
        B = (gstarts[graph_start[c]:graph_start[c + 1] + 1] - node_start[c]).astype(np.int64)
        ng = int(g_real[c])
        for gl in range(ng):
            plo[c, gl % 128, gl // 128] = B[gl]
            phi[c, gl % 128, gl // 128] = B[gl + 1]
            n = int(B[gl + 1] - B[gl])
            pscale[c, gl % 128, gl // 128] = 1.0 / max(n, 1)

    # ---- shared weights, packed for the device ----
    embW = np.asarray(embW, np.float32)
    embB = np.asarray(embB, np.float32)
    bondW = np.asarray(bondW, np.float32)
    bondB = np.asarray(bondB, np.float32)
    W1 = np.asarray(W1, np.float32)
    W2 = np.asarray(W2, np.float32)
    g1 = np.asarray(g1, np.float32)
    be1 = np.asarray(be1, np.float32)
    gout = np.asarray(gout, np.float32)
    bout = np.asarray(bout, np.float32)

    embW_aug = np.concatenate([embW, embB[None, :]], axis=0)          # [ATOM+1, D]
    bondW_aug = np.zeros((BOND + 1, L * D), np.float32)
    for i in range(L):
        bondW_aug[:BOND, i * D:(i + 1) * D] = bondW[i]
        bondW_aug[BOND, i * D:(i + 1) * D] = bondB[i]
    W1s = np.concatenate([W1[i] for i in range(L)], axis=1)           # [D, L*2D]
    W2s = np.zeros((128, L * 2 * 128), np.float32)
    for i in range(L):
        for h in range(2):
            W2s[:, (i * 2 + h) * 128:(i * 2 + h + 1) * 128] = W2[i][h * 128:(h + 1) * 128, :]
    g1p = np.zeros((128, 2 * L), np.float32)
    be1p = np.zeros((128, 2 * L), np.float32)
    for i in range(L):
        for h in range(2):
            g1p[:, i * 2 + h] = g1[i, h * 128:(h + 1) * 128]
            be1p[:, i * 2 + h] = be1[i, h * 128:(h + 1) * 128]
    goutp = gout.T.copy()                                             # [128, L]
    boutp = bout.T.copy()
    iota = np.tile(np.arange(128, dtype=np.float32), (128, 1))
    ident = np.eye(128, dtype=np.float32)

    cfg = dict(N=N, E=E, D=int(D), L=int(L), ATOM=ATOM, BOND=BOND, G=G,
               Npad=Npad, NW=NW, NT=NT, NSB=NSB, CPW=CPW, EC=EC, ES=ES,
               Gpad=Gpad, NPW=NPW,
               node_start=node_start, graph_start=graph_start,
               n_real=n_real, g_real=g_real)
    shared = dict(embW_aug=embW_aug, bondW_aug=bondW_aug, W1s=W1s, W2s=W2s,
                  g1p=g1p, be1p=be1p, goutp=goutp, boutp=boutp,
                  iota=iota, ident=ident)
    per_core = dict(xT=xT, eattrT=eattrT, gidx=gidx, dstin=dstin, kc=kc,
                    plo=plo, phi=phi, pscale=pscale)
    return cfg, shared, per_core


def build(cfg):
    import concourse.bacc as bacc
    import concourse.mybir as mybir
    import concourse.tile as tile
    from concourse.bass import IndirectOffsetOnAxis

    f32 = mybir.dt.float32
    i32 = mybir.dt.int32
    AF = mybir.ActivationFunctionType
    OP = mybir.AluOpType

    D = cfg["D"]; L = cfg["L"]; ATOM = cfg["ATOM"]; BOND = cfg["BOND"]
    Npad = cfg["Npad"]; NW = cfg["NW"]; NT = cfg["NT"]; NSB = cfg["NSB"]
    CPW = cfg["CPW"]; EC = cfg["EC"]; ES = cfg["ES"]
    Gpad = cfg["Gpad"]; NPW = cfg["NPW"]
    invN = 1.0 / float(cfg["N"])

    nc = bacc.Bacc("TRN2", target_bir_lowering=False, debug=False, num_devices=M)

    P_xT = nc.declare_dram_parameter("xT", [ATOM + 1, Npad], f32, isOutput=False)
    P_ea = nc.declare_dram_parameter("eattrT", [BOND + 1, ES], f32, isOutput=False)
    P_gi = nc.declare_dram_parameter("gidx", [128, EC], i32, isOutput=False)
    P_di = nc.declare_dram_parameter("dstin", [128, EC], f32, isOutput=False)
    P_kc = nc.declare_dram_parameter("kc", [128, 1], f32, isOutput=False)
    P_plo = nc.declare_dram_parameter("plo", [128, NPW], i32, isOutput=False)
    P_phi = nc.declare_dram_parameter("phi", [128, NPW], i32, isOutput=False)
    P_ps = nc.declare_dram_parameter("pscale", [128, NPW], f32, isOutput=False)
    P_embW = nc.declare_dram_parameter("embW_aug", [ATOM + 1, D], f32, isOutput=False)
    P_bondW = nc.declare_dram_parameter("bondW_aug", [BOND + 1, L * D], f32, isOutput=False)
    P_W1 = nc.declare_dram_parameter("W1s", [D, L * 2 * D], f32, isOutput=False)
    P_W2 = nc.declare_dram_parameter("W2s", [128, L * 2 * 128], f32, isOutput=False)
    P_g1 = nc.declare_dram_parameter("g1p", [128, 2 * L], f32, isOutput=False)
    P_be1 = nc.declare_dram_parameter("be1p", [128, 2 * L], f32, isOutput=False)
    P_gout = nc.declare_dram_parameter("goutp", [128, L], f32, isOutput=False)
    P_bout = nc.declare_dram_parameter("boutp", [128, L], f32, isOutput=False)
    P_iota = nc.declare_dram_parameter("iota", [128, 128], f32, isOutput=False)
    P_ident = nc.declare_dram_parameter("ident", [128, 128], f32, isOutput=False)
    P_out = nc.declare_dram_parameter("out", [Gpad, D], f32, isOutput=True)

    with tile.TileContext(nc) as tc:
        with tc.tile_pool(name="const", bufs=1) as cp, \
             tc.tile_pool(name="state", bufs=1) as statep, \
             tc.tile_pool(name="xin", bufs=3) as xinp, \
             tc.tile_pool(name="ein", bufs=2) as einp, \
             tc.tile_pool(name="gat", bufs=2) as gatp, \
             tc.tile_pool(name="sm", bufs=4) as smp, \
             tc.tile_pool(name="z1n", bufs=3) as z1np, \
             tc.tile_pool(name="scr", bufs=2) as scrp, \
             tc.tile_pool(name="st", bufs=2) as stp, \
             tc.tile_pool(name="pse", bufs=2, space="PSUM") as pse, \
             tc.tile_pool(name="pagg", bufs=2, space="PSUM") as pagg, \
             tc.tile_pool(name="pnode", bufs=2, space="PSUM") as pnode, \
             tc.tile_pool(name="pacc", bufs=2, space="PSUM") as pacc, \
             tc.tile_pool(name="dram", bufs=1, space="DRAM") as dp:

            # ---------- constants ----------
            embW_t = cp.tile([ATOM + 1, D], f32, name="embW_t")
            bondW_t = cp.tile([BOND + 1, L * D], f32, name="bondW_t")
            W1_t = cp.tile([D, L * 2 * D], f32, name="W1_t")
            W2_t = cp.tile([128, L * 2 * 128], f32, name="W2_t")
            g1_t = cp.tile([128, 2 * L], f32, name="g1_t")
            be1_t = cp.tile([128, 2 * L], f32, name="be1_t")
            gout_t = cp.tile([128, L], f32, name="gout_t")
            bout_t = cp.tile([128, L], f32, name="bout_t")
            iota_t = cp.tile([128, 128], f32, name="iota_t")
            ident_t = cp.tile([128, 128], f32, name="ident_t")
            kc_t = cp.tile([128, 1], f32, name="kc_t")
            eps_t = cp.tile([128, 1], f32, name="eps_t")
            gidx_t = cp.tile([128, EC], i32, name="gidx_t")
            dstin_t = cp.tile([128, EC], f32, name="dstin_t")
            plo_t = cp.tile([128, NPW], i32, name="plo_t")
            phi_t = cp.tile([128, NPW], i32, name="phi_t")
            pscale_t = cp.tile([128, NPW], f32, name="pscale_t")
            for t, pr in [(embW_t, P_embW), (bondW_t, P_bondW), (W1_t, P_W1),
                          (W2_t, P_W2), (g1_t, P_g1), (be1_t, P_be1),
                          (gout_t, P_gout), (bout_t, P_bout), (iota_t, P_iota),
                          (ident_t, P_ident), (kc_t, P_kc), (gidx_t, P_gi),
                          (dstin_t, P_di), (plo_t, P_plo), (phi_t, P_phi),
                          (pscale_t, P_ps)]:
                nc.sync.dma_start(out=t[:], in_=pr[:])

            nc.vector.memset(eps_t[:], BN_EPS)
            hT = statep.tile([128, Npad], f32, name="hT")
            h2T = statep.tile([128, Npad], f32, name="h2T")
            h_rows = dp.tile([Npad + 128, D], f32, name="h_rows")
            h_tables = [dp.tile([M * Npad, D], f32, addr_space="Shared", name=f"h_table{k}")
                        for k in range(L)]
            ar_ins = [dp.tile([128, 4], f32, name=f"ar_in{k}") for k in range(2 * L)]
            ar_outs = [dp.tile([128, 4], f32, addr_space="Shared", name=f"ar_out{k}")
                       for k in range(2 * L)]

            def transpose_to_rows(src_tile, col_lo, n_cols, row_off):
                # src_tile[:, col_lo : col_lo + n_cols] -> h_rows[row_off + ...]
                for c in range(n_cols // 128):
                    ps = pse.tile([128, 128], f32, name="ps_tr", tag="ps_tr")
                    nc.tensor.transpose(out=ps[:], in_=src_tile[:, col_lo + c * 128: col_lo + (c + 1) * 128],
                                        identity=ident_t[:])
                    rsb = smp.tile([128, 128], f32, name="rsb", tag="rsb")
                    nc.scalar.activation(out=rsb[:], in_=ps[:], func=AF.Copy)
                    nc.sync.dma_start(out=h_rows[row_off + c * 128: row_off + (c + 1) * 128, :],
                                      in_=rsb[:])

            # ---------- embedding: hT = (x_aug @ embW_aug)^T ----------
            for t in range(NT):
                xt = xinp.tile([ATOM + 1, 512], f32, name="xt", tag="xt")
                nc.sync.dma_start(out=xt[:], in_=P_xT[:, t * 512:(t + 1) * 512])
                ps = pnode.tile([128, 512], f32, name="ps_emb", tag="ps_node")
                nc.tensor.matmul(out=ps[:], lhsT=embW_t[:], rhs=xt[:], start=True, stop=True)
                nc.scalar.activation(out=hT[:, t * 512:(t + 1) * 512], in_=ps[:], func=AF.Copy)
            transpose_to_rows(hT, 0, Npad, 0)
            nc.gpsimd.collective_compute(
                "AllGather", OP.bypass, replica_groups=[list(range(M))],
                ins=[h_rows[0:Npad, :].opt()], outs=[h_tables[0][:].opt()])

            stats_small = []

            def small(name):
                t = stp.tile([128, 1], f32, name=name, tag=name)
                return t

            for i in range(L):
                # ================= edge phase =================
                for sb in range(NSB):
                    s_lo = sb * SBW * CPW * 128
                    s_hi = (sb + 1) * SBW * CPW * 128
                    ea = einp.tile([BOND + 1, SBW * CPW * 128], f32, name="ea", tag="ea")
                    nc.sync.dma_start(out=ea[:], in_=P_ea[:, s_lo:s_hi])
                    gat = gatp.tile([128, SBW * CPW * 128], f32, name="gat", tag="gat")
                    for wl in range(SBW):
                        w = sb * SBW + wl
                        ps_a = pagg.tile([128, 128], f32, name="ps_a", tag="ps_a")
                        for j in range(CPW):
                            ch = wl * CPW + j
                            gcol = w * CPW + j
                            nc.gpsimd.indirect_dma_start(
                                out=gat[:, ch * 128:(ch + 1) * 128], out_offset=None,
                                in_=h_tables[i][:],
                                in_offset=IndirectOffsetOnAxis(ap=gidx_t[:, gcol:gcol + 1], axis=0))
                            ps_e = pse.tile([128, 128], f32, name="ps_e", tag="ps_tr")
                            nc.tensor.matmul(out=ps_e[:], lhsT=ea[:, ch * 128:(ch + 1) * 128],
                                             rhs=bondW_t[:, i * D:(i + 1) * D],
                                             start=True, stop=False)
                            nc.tensor.matmul(out=ps_e[:], lhsT=ident_t[:],
                                             rhs=gat[:, ch * 128:(ch + 1) * 128],
                                             start=False, stop=True)
                            msg = smp.tile([128, 128], f32, name="msg", tag="msg")
                            nc.scalar.activation(out=msg[:], in_=ps_e[:], func=AF.Relu)
                            S = smp.tile([128, 128], f32, name="S", tag="S")
                            nc.vector.tensor_tensor(
                                out=S[:], in0=dstin_t[:, gcol:gcol + 1].to_broadcast([128, 128]),
                                in1=iota_t[:], op=OP.is_equal)
                            nc.tensor.matmul(out=ps_a[:], lhsT=msg[:], rhs=S[:],
                                             start=(j == 0), stop=(j == CPW - 1))
                        # z = h + agg (in place)
                        nc.vector.tensor_tensor(out=hT[:, w * 128:(w + 1) * 128],
                                                in0=hT[:, w * 128:(w + 1) * 128],
                                                in1=ps_a[:], op=OP.add)

                # ================= node phase =================
                # Pass A: stats of raw z1 = z @ W1 (no bias; bias shifts mean only,
                # and BN cancels additive bias, so we ignore b1/b2 entirely).
                s1 = [stp.tile([128, NT], f32, name=f"s1{h}", tag=f"s1{h}") for h in range(2)]
                q1 = [stp.tile([128, NT], f32, name=f"q1{h}", tag=f"q1{h}") for h in range(2)]
                padv = [small(f"padv{h}") for h in range(2)]
                for t in range(NT):
                    for h in range(2):
                        ps = pnode.tile([128, 512], f32, name="ps_n", tag="ps_node")
                        nc.tensor.matmul(out=ps[:],
                                         lhsT=W1_t[:, (i * 2 + h) * 128:(i * 2 + h + 1) * 128],
                                         rhs=hT[:, t * 512:(t + 1) * 512], start=True, stop=True)
                        nc.vector.tensor_reduce(out=s1[h][:, t:t + 1], in_=ps[:],
                                                axis=mybir.AxisListType.X, op=OP.add)
                        scr = scrp.tile([128, 512], f32, name="scr", tag="scr")
                        nc.scalar.activation(out=scr[:], in_=ps[:], func=AF.Square,
                                             accum_out=q1[h][:, t:t + 1])
                        if t == NT - 1:
                            nc.vector.tensor_copy(out=padv[h][:], in_=ps[:, 511:512])
                ar_sb = stp.tile([128, 4], f32, name="ar_sb", tag="ar_sb")
                tmp = small("tmp"); tmp2 = small("tmp2")
                for h in range(2):
                    nc.vector.tensor_reduce(out=ar_sb[:, 2 * h:2 * h + 1], in_=s1[h][:],
                                            axis=mybir.AxisListType.X, op=OP.add)
                    nc.vector.tensor_reduce(out=ar_sb[:, 2 * h + 1:2 * h + 2], in_=q1[h][:],
                                            axis=mybir.AxisListType.X, op=OP.add)
                    # subtract padding contribution: kc * v and kc * v^2
                    nc.vector.tensor_tensor(out=tmp[:], in0=padv[h][:], in1=kc_t[:], op=OP.mult)
                    nc.vector.tensor_tensor(out=ar_sb[:, 2 * h:2 * h + 1],
                                            in0=ar_sb[:, 2 * h:2 * h + 1], in1=tmp[:], op=OP.subtract)
                    nc.vector.tensor_tensor(out=tmp2[:], in0=padv[h][:], in1=padv[h][:], op=OP.mult)
                    nc.vector.tensor_tensor(out=tmp2[:], in0=tmp2[:], in1=kc_t[:], op=OP.mult)
                    nc.vector.tensor_tensor(out=ar_sb[:, 2 * h + 1:2 * h + 2],
                                            in0=ar_sb[:, 2 * h + 1:2 * h + 2], in1=tmp2[:], op=OP.subtract)
                nc.sync.dma_start(out=ar_ins[2 * i][:], in_=ar_sb[:])
                nc.gpsimd.collective_compute(
                    "AllReduce", OP.add, replica_groups=[list(range(M))],
                    ins=[ar_ins[2 * i][:].opt()], outs=[ar_outs[2 * i][:].opt()])
                st1 = stp.tile([128, 4], f32, name="st1", tag="st1")
                nc.sync.dma_start(out=st1[:], in_=ar_outs[2 * i][:])

                A1 = [small(f"A1{h}") for h in range(2)]
                B1 = [small(f"B1{h}") for h in range(2)]
                mny = small("mny"); mq = small("mq"); var = small("var"); sd = small("sd")
                for h in range(2):
                    nc.vector.tensor_scalar_mul(out=mny[:], in0=st1[:, 2 * h:2 * h + 1], scalar1=invN)
                    nc.vector.tensor_scalar_mul(out=mq[:], in0=st1[:, 2 * h + 1:2 * h + 2], scalar1=invN)
                    nc.vector.tensor_tensor(out=var[:], in0=mny[:], in1=mny[:], op=OP.mult)
                    nc.vector.tensor_tensor(out=var[:], in0=mq[:], in1=var[:], op=OP.subtract)
                    nc.scalar.activation(out=sd[:], in_=var[:], func=AF.Sqrt, bias=eps_t[:, :1])
                    nc.vector.reciprocal(out=A1[h][:], in_=sd[:])
                    nc.vector.tensor_tensor(out=A1[h][:], in0=A1[h][:],
                                            in1=g1_t[:, i * 2 + h:i * 2 + h + 1], op=OP.mult)
                    nc.vector.tensor_tensor(out=B1[h][:], in0=mny[:], in1=A1[h][:], op=OP.mult)
                    nc.vector.tensor_tensor(out=B1[h][:], in0=be1_t[:, i * 2 + h:i * 2 + h + 1],
                                            in1=B1[h][:], op=OP.subtract)

                # Pass B: recompute z1, apply BN1+relu, z2 = z1n @ W2, stats of h_pre
                s2 = stp.tile([128, NT], f32, name="s2", tag="s2")
                q2 = stp.tile([128, NT], f32, name="q2", tag="q2")
                padv2 = small("padv2")
                for t in range(NT):
                    ps2 = pacc.tile([128, 512], f32, name="ps2", tag="ps2")
                    for h in range(2):
                        ps = pnode.tile([128, 512], f32, name="ps_n", tag="ps_node")
                        nc.tensor.matmul(out=ps[:],
                                         lhsT=W1_t[:, (i * 2 + h) * 128:(i * 2 + h + 1) * 128],
                                         rhs=hT[:, t * 512:(t + 1) * 512], start=True, stop=True)
                        z1n = z1np.tile([128, 512], f32, name="z1n", tag="z1n")
                        nc.scalar.activation(out=z1n[:], in_=ps[:], func=AF.Relu,
                                             scale=A1[h][:, :1], bias=B1[h][:, :1])
                        nc.tensor.matmul(out=ps2[:],
                                         lhsT=W2_t[:, (i * 2 + h) * 128:(i * 2 + h + 1) * 128],
                                         rhs=z1n[:], start=(h == 0), stop=(h == 1))
                    nc.scalar.activation(out=h2T[:, t * 512:(t + 1) * 512], in_=ps2[:],
                                         func=AF.Copy, accum_out=s2[:, t:t + 1])
                    scr = scrp.tile([128, 512], f32, name="scr", tag="scr")
                    nc.scalar.activation(out=scr[:], in_=h2T[:, t * 512:(t + 1) * 512],
                                         func=AF.Square, accum_out=q2[:, t:t + 1])
                    if t == NT - 1:
                        nc.vector.tensor_copy(out=padv2[:], in_=ps2[:, 511:512])
                ar_sb2 = stp.tile([128, 4], f32, name="ar_sb2", tag="ar_sb")
                nc.vector.memset(ar_sb2[:, 2:4], 0.0)
                nc.vector.tensor_reduce(out=ar_sb2[:, 0:1], in_=s2[:],
                                        axis=mybir.AxisListType.X, op=OP.add)
                nc.vector.tensor_reduce(out=ar_sb2[:, 1:2], in_=q2[:],
                                        axis=mybir.AxisListType.X, op=OP.add)
                nc.vector.tensor_tensor(out=tmp[:], in0=padv2[:], in1=kc_t[:], op=OP.mult)
                nc.vector.tensor_tensor(out=ar_sb2[:, 0:1], in0=ar_sb2[:, 0:1], in1=tmp[:], op=OP.subtract)
                nc.vector.tensor_tensor(out=tmp2[:], in0=padv2[:], in1=padv2[:], op=OP.mult)
                nc.vector.tensor_tensor(out=tmp2[:], in0=tmp2[:], in1=kc_t[:], op=OP.mult)
                nc.vector.tensor_tensor(out=ar_sb2[:, 1:2], in0=ar_sb2[:, 1:2], in1=tmp2[:], op=OP.subtract)
                nc.sync.dma_start(out=ar_ins[2 * i + 1][:], in_=ar_sb2[:])
                nc.gpsimd.collective_compute(
                    "AllReduce", OP.add, replica_groups=[list(range(M))],
                    ins=[ar_ins[2 * i + 1][:].opt()], outs=[ar_outs[2 * i + 1][:].opt()])
                st2 = stp.tile([128, 4], f32, name="st2", tag="st1")
                nc.sync.dma_start(out=st2[:], in_=ar_outs[2 * i + 1][:])

                A2 = small("A2"); B2 = small("B2")
                nc.vector.tensor_scalar_mul(out=mny[:], in0=st2[:, 0:1], scalar1=invN)
                nc.vector.tensor_scalar_mul(out=mq[:], in0=st2[:, 1:2], scalar1=invN)
                nc.vector.tensor_tensor(out=var[:], in0=mny[:], in1=mny[:], op=OP.mult)
                nc.vector.tensor_tensor(out=var[:], in0=mq[:], in1=var[:], op=OP.subtract)
                nc.scalar.activation(out=sd[:], in_=var[:], func=AF.Sqrt, bias=eps_t[:, :1])
                nc.vector.reciprocal(out=A2[:], in_=sd[:])
                nc.vector.tensor_tensor(out=A2[:], in0=A2[:], in1=gout_t[:, i:i + 1], op=OP.mult)
                nc.vector.tensor_tensor(out=B2[:], in0=mny[:], in1=A2[:], op=OP.mult)
                nc.vector.tensor_tensor(out=B2[:], in0=bout_t[:, i:i + 1], in1=B2[:], op=OP.subtract)

                # Pass C: h = act(h_pre * A2 + B2); rows + AllGather for next layer
                fn = AF.Relu if i < L - 1 else AF.Identity
                for t in range(NT):
                    nc.scalar.activation(out=hT[:, t * 512:(t + 1) * 512],
                                         in_=h2T[:, t * 512:(t + 1) * 512],
                                         func=fn, scale=A2[:, :1], bias=B2[:, :1])
                if i < L - 1:
                    transpose_to_rows(hT, 0, Npad, 0)
                    nc.gpsimd.collective_compute(
                        "AllGather", OP.bypass, replica_groups=[list(range(M))],
                        ins=[h_rows[0:Npad, :].opt()], outs=[h_tables[i + 1][:].opt()])

            # ================= pooling =================
            # exclusive prefix sums along nodes, stored as rows; pooled mean by
            # two indirect row gathers (lo/hi) per 128-graph window.
            pref = h2T
            carry = stp.tile([128, 1], f32, name="carry", tag="carry")
            nc.vector.memset(carry[:], 0.0)
            for t in range(NT):
                nc.vector.tensor_tensor_scan(
                    out=pref[:, t * 512:(t + 1) * 512],
                    data0=hT[:, t * 512:(t + 1) * 512],
                    data1=hT[:, t * 512:(t + 1) * 512],
                    initial=carry[:, :1] if t > 0 else 0.0,
                    op0=OP.add, op1=OP.bypass)
                if t < NT - 1:
                    nc.vector.tensor_copy(out=carry[:], in_=pref[:, (t + 1) * 512 - 1:(t + 1) * 512])
            # write exclusive-prefix rows: h_rows[0] = 0, h_rows[r] = pref[:, r-1]
            zrow = stp.tile([128, 1], f32, name="zrow", tag="zrow")
            nc.vector.memset(zrow[:], 0.0)
            zsb = smp.tile([128, 128], f32, name="zsb", tag="rsb")
            nc.vector.memset(zsb[:], 0.0)
            nc.sync.dma_start(out=h_rows[0:1, :], in_=zsb[:1, :])
            transpose_to_rows(pref, 0, Npad, 1)
            from concourse.bass import IndirectOffsetOnAxis as IOA
            for pw in range(NPW):
                glo = smp.tile([128, D], f32, name="glo", tag="glo")
                ghi = smp.tile([128, D], f32, name="ghi", tag="ghi")
                nc.gpsimd.indirect_dma_start(out=glo[:], out_offset=None, in_=h_rows[:],
                                             in_offset=IOA(ap=plo_t[:, pw:pw + 1], axis=0))
                nc.gpsimd.indirect_dma_start(out=ghi[:], out_offset=None, in_=h_rows[:],
                                             in_offset=IOA(ap=phi_t[:, pw:pw + 1], axis=0))
                osb = smp.tile([128, D], f32, name="osb", tag="osb")
                nc.vector.tensor_tensor(out=osb[:], in0=ghi[:], in1=glo[:], op=OP.subtract)
                nc.vector.tensor_scalar_mul(out=osb[:], in0=osb[:], scalar1=pscale_t[:, pw:pw + 1])
                nc.sync.dma_start(out=P_out[pw * 128:(pw + 1) * 128, :], in_=osb[:])

    nc.finalize()
    return nc


def kernel(_G=G_DEFAULT, _trace=False, **inputs):
    _register_ntff_hook()
    from concourse.bass_utils import run_bass_kernel_spmd

    cfg, shared, per_core = preprocess(G=_G, **inputs)
    nc = build(cfg)

    in_maps = []
    for c in range(M):
        m = dict(shared)
        m["xT"] = per_core["xT"][c]
        m["eattrT"] = per_core["eattrT"][c]
        m["gidx"] = per_core["gidx"][c]
        m["dstin"] = per_core["dstin"][c]
        m["kc"] = per_core["kc"][c]
        m["plo"] = per_core["plo"][c]
        m["phi"] = per_core["phi"][c]
        m["pscale"] = per_core["pscale"][c]
        in_maps.append(m)

    res = run_bass_kernel_spmd(nc, in_maps, list(range(M)), trace=_trace)

    G = cfg["G"]; D = cfg["D"]
    out = np.zeros((G, D), np.float32)
    gs = cfg["graph_start"]
    for c in range(M):
        ng = int(cfg["g_real"][c])
        out[gs[c]:gs[c] + ng] = res.results[c]["out"][:ng]
    kernel._last_exec_ns = res.exec_time_ns
    return out


# revision 8
# speedup vs baseline: 1.2916x; 1.2916x over previous
"""Trainium2 Bass kernel for AtomGraphGINE message passing (8 NeuronCores).

Distribution: nodes+edges sharded by graph (batch is sorted, shards are graph
aligned). Weights replicated. Per layer: AllGather of node states h into a
replicated DRAM table, per-edge rows gathered with indirect DMA, message
relu(h[src]+e) formed in PSUM, scatter-add to destination nodes via one-hot
matmuls accumulated per 128-node window, dense node update with BatchNorm
(global stats via small AllReduce, padding corrected analytically), final
per-graph mean pooling via exclusive prefix sums and two indirect gathers.
"""

import sys
import types

import numpy as np

M = 8          # NeuronCores
G_DEFAULT = 4096
BN_EPS = 1e-5
SBW = 4        # windows per gather superbatch (SBUF staging granularity)


def _register_ntff_hook():
    if "antenv.axon_hooks" in sys.modules:
        return
    try:
        import antenv
    except ImportError:
        return
    mod = types.ModuleType("antenv.axon_hooks")
    mod._hook = None

    def set_axon_ntff_profile_hook(h):
        mod._hook = h

    def get_axon_ntff_profile_hook():
        return mod._hook

    mod.set_axon_ntff_profile_hook = set_axon_ntff_profile_hook
    mod.get_axon_ntff_profile_hook = get_axon_ntff_profile_hook
    sys.modules["antenv.axon_hooks"] = mod
    antenv.axon_hooks = mod
    try:
        from trn_agent_boot.trn_boot import _ntff_profile_via_ctypes
        set_axon_ntff_profile_hook(_ntff_profile_via_ctypes("/opt/axon/libaxon_pjrt.so"))
    except Exception:
        pass


def _round_up(x, m):
    return int((x + m - 1) // m) * m


def preprocess(x, edge_attr, embW, embB, bondW, bondB, W1, b1, g1, be1,
               W2, b2, gout, bout, edge_index, batch, G):
    x = np.asarray(x, np.float32)
    edge_attr = np.asarray(edge_attr, np.float32)
    src = np.asarray(edge_index[0], np.int64)
    dst = np.asarray(edge_index[1], np.int64)
    batch = np.asarray(batch, np.int64)
    N, ATOM = x.shape
    E = src.shape[0]
    BOND = edge_attr.shape[1]
    D = np.asarray(embW).shape[1]
    L = np.asarray(bondW).shape[0]

    # ---- graph-aligned node partition over M cores ----
    gstarts = np.searchsorted(batch, np.arange(G + 1))  # node start of each graph
    ideal = (np.arange(M + 1) * N) // M
    gsel = np.searchsorted(gstarts, ideal)
    gsel = np.clip(gsel, 0, G)
    gsel[0], gsel[M] = 0, G
    for c in range(1, M):  # snap to nearest boundary, keep monotone
        lo = max(gsel[c] - 1, gsel[c - 1] + 1)
        hi = min(gsel[c] + 1, gsel[c + 1] - 1) if c < M else gsel[c]
        best, bestd = gsel[c], abs(int(gstarts[gsel[c]]) - int(ideal[c]))
        for g in range(lo, hi + 1):
            d = abs(int(gstarts[g]) - int(ideal[c]))
            if d < bestd:
                best, bestd = g, d
        gsel[c] = best
    graph_start = gsel.astype(np.int64)
    node_start = gstarts[graph_start].astype(np.int64)
    n_real = np.diff(node_start)
    g_real = np.diff(graph_start)
    assert (n_real > 0).all()

    Npad = _round_up(int(n_real.max()) + 1, 512)
    NW = Npad // 128
    NT = Npad // 512
    NSB = NW // SBW

    # ---- edge partition by dst owner; window = dst_local // 128 ----
    owner = np.searchsorted(node_start, dst, side="right") - 1
    dst_local = dst - node_start[owner]
    w_dst = dst_local // 128
    srco = np.searchsorted(node_start, src, side="right") - 1
    grow = srco * Npad + (src - node_start[srco])  # row in the AG'd table

    key = owner * NW + w_dst
    cnt = np.bincount(key, minlength=M * NW)
    CPW = max(1, int(-(-int(cnt.max()) // 128)))
    SLOTW = CPW * 128
    EC = NW * CPW            # index columns per core
    ES = NW * SLOTW          # edge slots per core

    order = np.lexsort((grow, key))
    key_s = key[order]
    grow_s = grow[order]
    dstl_s = dst_local[order]
    ea_s = edge_attr[order]
    # position within each (owner, window) group
    firsts = np.searchsorted(key_s, np.arange(M * NW))
    within = np.arange(E) - firsts[key_s]
    slot = (key_s % NW) * SLOTW + within          # slot within the core
    core_e = key_s // NW
    p = slot % 128
    col = slot // 128

    gidx = np.zeros((M, 128, EC), np.int32)
    dstin = np.full((M, 128, EC), -1.0, np.float32)
    eattrT = np.zeros((M, BOND + 1, ES), np.float32)
    gidx[core_e, p, col] = grow_s.astype(np.int32)
    dstin[core_e, p, col] = (dstl_s % 128).astype(np.float32)
    eattrT[core_e[:, None], np.arange(BOND)[None, :], slot[:, None]] = ea_s
    eattrT[core_e, BOND, slot] = 1.0

    # ---- per-core transposed, padded node features ----
    xT = np.zeros((M, ATOM + 1, Npad), np.float32)
    for c in range(M):
        xs = x[node_start[c]:node_start[c + 1]]
        xT[c, :ATOM, :n_real[c]] = xs.T
        xT[c, ATOM, :n_real[c]] = 1.0

    kc = np.zeros((M, 128, 1), np.float32)
    for c in range(M):
        kc[c, :, 0] = float(Npad - n_real[c])

    # ---- pooling tables (exclusive prefix rows) ----
    Gpad = _round_up(max(int(g_real.max()), 1), 128)
    NPW = Gpad // 128
    plo = np.zeros((M, 128, NPW), np.int32)
    phi = np.zeros((M, 128, NPW), np.int32)
    pscale = np.zeros((M, 128, NPW), np.float32)
    for c in range(M):
        B = (gstarts[graph_start[c]:graph_start[c + 1] + 1] - node_start[c]).astype(np.int64)
        ng = int(g_real[c])
        for gl in range(ng):
            plo[c, gl % 128, gl // 128] = B[gl]
            phi[c, gl % 128, gl // 128] = B[gl + 1]
            n = int(B[gl + 1] - B[gl])
            pscale[c, gl % 128, gl // 128] = 1.0 / max(n, 1)

    # ---- shared weights, packed for the device ----
    embW = np.asarray(embW, np.float32)
    embB = np.asarray(embB, np.float32)
    bondW = np.asarray(bondW, np.float32)
    bondB = np.asarray(bondB, np.float32)
    W1 = np.asarray(W1, np.float32)
    W2 = np.asarray(W2, np.float32)
    g1 = np.asarray(g1, np.float32)
    be1 = np.asarray(be1, np.float32)
    gout = np.asarray(gout, np.float32)
    bout = np.asarray(bout, np.float32)

    embW_aug = np.concatenate([embW, embB[None, :]], axis=0)          # [ATOM+1, D]
    bondW_aug = np.zeros((BOND + 1, L * D), np.float32)
    for i in range(L):
        bondW_aug[:BOND, i * D:(i + 1) * D] = bondW[i]
        bondW_aug[BOND, i * D:(i + 1) * D] = bondB[i]
    W1s = np.concatenate([W1[i] for i in range(L)], axis=1)           # [D, L*2D]
    W2s = np.zeros((128, L * 2 * 128), np.float32)
    for i in range(L):
        for h in range(2):
            W2s[:, (i * 2 + h) * 128:(i * 2 + h + 1) * 128] = W2[i][h * 128:(h + 1) * 128, :]
    g1p = np.zeros((128, 2 * L), np.float32)
    be1p = np.zeros((128, 2 * L), np.float32)
    for i in range(L):
        for h in range(2):
            g1p[:, i * 2 + h] = g1[i, h * 128:(h + 1) * 128]
            be1p[:, i * 2 + h] = be1[i, h * 128:(h + 1) * 128]
    goutp = gout.T.copy()                                             # [128, L]
    boutp = bout.T.copy()
    iota = np.tile(np.arange(128, dtype=np.float32), (128, 1))
    ident = np.eye(128, dtype=np.float32)

    cfg = dict(N=N, E=E, D=int(D), L=int(L), ATOM=ATOM, BOND=BOND, G=G,
               Npad=Npad, NW=NW, NT=NT, NSB=NSB, CPW=CPW, EC=EC, ES=ES,
               Gpad=Gpad, NPW=NPW,
               node_start=node_start, graph_start=graph_start,
               n_real=n_real, g_real=g_real)
    shared = dict(embW_aug=embW_aug, bondW_aug=bondW_aug, W1s=W1s, W2s=W2s,
                  g1p=g1p, be1p=be1p, goutp=goutp, boutp=boutp,
                  iota=iota, ident=ident)
    per_core = dict(xT=xT, eattrT=eattrT, gidx=gidx, dstin=dstin, kc=kc,
                    plo=plo, phi=phi, pscale=pscale)
    return cfg, shared, per_core


def build(cfg):
    import concourse.bacc as bacc
    import concourse.mybir as mybir
    import concourse.tile as tile
    from concourse.bass import IndirectOffsetOnAxis

    f32 = mybir.dt.float32
    bf16 = mybir.dt.bfloat16
    i32 = mybir.dt.int32
    AF = mybir.ActivationFunctionType
    OP = mybir.AluOpType

    D = cfg["D"]; L = cfg["L"]; ATOM = cfg["ATOM"]; BOND = cfg["BOND"]
    Npad = cfg["Npad"]; NW = cfg["NW"]; NT = cfg["NT"]; NSB = cfg["NSB"]
    CPW = cfg["CPW"]; EC = cfg["EC"]; ES = cfg["ES"]
    Gpad = cfg["Gpad"]; NPW = cfg["NPW"]
    invN = 1.0 / float(cfg["N"])

    nc = bacc.Bacc("TRN2", target_bir_lowering=False, debug=False, num_devices=M)

    P_xT = nc.declare_dram_parameter("xT", [ATOM + 1, Npad], f32, isOutput=False)
    P_ea = nc.declare_dram_parameter("eattrT", [BOND + 1, ES], f32, isOutput=False)
    P_gi = nc.declare_dram_parameter("gidx", [128, EC], i32, isOutput=False)
    P_di = nc.declare_dram_parameter("dstin", [128, EC], f32, isOutput=False)
    P_kc = nc.declare_dram_parameter("kc", [128, 1], f32, isOutput=False)
    P_plo = nc.declare_dram_parameter("plo", [128, NPW], i32, isOutput=False)
    P_phi = nc.declare_dram_parameter("phi", [128, NPW], i32, isOutput=False)
    P_ps = nc.declare_dram_parameter("pscale", [128, NPW], f32, isOutput=False)
    P_embW = nc.declare_dram_parameter("embW_aug", [ATOM + 1, D], f32, isOutput=False)
    P_bondW = nc.declare_dram_parameter("bondW_aug", [BOND + 1, L * D], f32, isOutput=False)
    P_W1 = nc.declare_dram_parameter("W1s", [D, L * 2 * D], f32, isOutput=False)
    P_W2 = nc.declare_dram_parameter("W2s", [128, L * 2 * 128], f32, isOutput=False)
    P_g1 = nc.declare_dram_parameter("g1p", [128, 2 * L], f32, isOutput=False)
    P_be1 = nc.declare_dram_parameter("be1p", [128, 2 * L], f32, isOutput=False)
    P_gout = nc.declare_dram_parameter("goutp", [128, L], f32, isOutput=False)
    P_bout = nc.declare_dram_parameter("boutp", [128, L], f32, isOutput=False)
    P_iota = nc.declare_dram_parameter("iota", [128, 128], f32, isOutput=False)
    P_ident = nc.declare_dram_parameter("ident", [128, 128], f32, isOutput=False)
    P_out = nc.declare_dram_parameter("out", [Gpad, D], f32, isOutput=True)

    with tile.TileContext(nc) as tc:
        with tc.tile_pool(name="const", bufs=1) as cp, \
             tc.tile_pool(name="state", bufs=1) as statep, \
             tc.tile_pool(name="xin", bufs=3) as xinp, \
             tc.tile_pool(name="ein", bufs=2) as einp, \
             tc.tile_pool(name="gat", bufs=2) as gatp, \
             tc.tile_pool(name="sm", bufs=4) as smp, \
             tc.tile_pool(name="z1n", bufs=3) as z1np, \
             tc.tile_pool(name="scr", bufs=2) as scrp, \
             tc.tile_pool(name="st", bufs=2) as stp, \
             tc.tile_pool(name="pse", bufs=2, space="PSUM") as pse, \
             tc.tile_pool(name="pagg", bufs=2, space="PSUM") as pagg, \
             tc.tile_pool(name="pnode", bufs=2, space="PSUM") as pnode, \
             tc.tile_pool(name="pacc", bufs=2, space="PSUM") as pacc, \
             tc.tile_pool(name="dram", bufs=1, space="DRAM") as dp:

            # ---------- constants ----------
            embW_t = cp.tile([ATOM + 1, D], f32, name="embW_t")
            bondW_t = cp.tile([BOND + 1, L * D], f32, name="bondW_t")
            W1_t = cp.tile([D, L * 2 * D], f32, name="W1_t")
            W2_t = cp.tile([128, L * 2 * 128], f32, name="W2_t")
            g1_t = cp.tile([128, 2 * L], f32, name="g1_t")
            be1_t = cp.tile([128, 2 * L], f32, name="be1_t")
            gout_t = cp.tile([128, L], f32, name="gout_t")
            bout_t = cp.tile([128, L], f32, name="bout_t")
            iota_t = cp.tile([128, 128], f32, name="iota_t")
            ident_t = cp.tile([128, 128], f32, name="ident_t")
            kc_t = cp.tile([128, 1], f32, name="kc_t")
            eps_t = cp.tile([128, 1], f32, name="eps_t")
            gidx_t = cp.tile([128, EC], i32, name="gidx_t")
            dstin_t = cp.tile([128, EC], f32, name="dstin_t")
            plo_t = cp.tile([128, NPW], i32, name="plo_t")
            phi_t = cp.tile([128, NPW], i32, name="phi_t")
            pscale_t = cp.tile([128, NPW], f32, name="pscale_t")
            for t, pr in [(embW_t, P_embW), (bondW_t, P_bondW), (W1_t, P_W1),
                          (W2_t, P_W2), (g1_t, P_g1), (be1_t, P_be1),
                          (gout_t, P_gout), (bout_t, P_bout), (iota_t, P_iota),
                          (ident_t, P_ident), (kc_t, P_kc), (gidx_t, P_gi),
                          (dstin_t, P_di), (plo_t, P_plo), (phi_t, P_phi),
                          (pscale_t, P_ps)]:
                nc.sync.dma_start(out=t[:], in_=pr[:])

            nc.vector.memset(eps_t[:], BN_EPS)
            hT = statep.tile([128, Npad], f32, name="hT")
            h2T = statep.tile([128, Npad], f32, name="h2T")
            h_rows = dp.tile([Npad + 128, D], f32, name="h_rows")
            h_rows_bf = dp.tile([Npad, D], bf16, name="h_rows_bf")
            h_tables = [dp.tile([M * Npad, D], bf16, addr_space="Shared", name=f"h_table{k}")
                        for k in range(L)]
            ar_ins = [dp.tile([128, 4], f32, name=f"ar_in{k}") for k in range(2 * L)]
            ar_outs = [dp.tile([128, 4], f32, addr_space="Shared", name=f"ar_out{k}")
                       for k in range(2 * L)]

            def transpose_to_rows(src_tile, col_lo, n_cols, row_off, bf=False):
                dst = h_rows_bf if bf else h_rows
                dt = bf16 if bf else f32
                for c in range(n_cols // 128):
                    ps = pse.tile([128, 128], f32, name="ps_tr", tag="ps_tr")
                    nc.tensor.transpose(out=ps[:], in_=src_tile[:, col_lo + c * 128: col_lo + (c + 1) * 128],
                                        identity=ident_t[:])
                    rsb = smp.tile([128, 128], dt, name="rsb", tag="rsb")
                    nc.scalar.activation(out=rsb[:], in_=ps[:], func=AF.Copy)
                    nc.sync.dma_start(out=dst[row_off + c * 128: row_off + (c + 1) * 128, :],
                                      in_=rsb[:])

            # ---------- embedding: hT = (x_aug @ embW_aug)^T ----------
            for t in range(NT):
                xt = xinp.tile([ATOM + 1, 512], f32, name="xt", tag="xt")
                nc.sync.dma_start(out=xt[:], in_=P_xT[:, t * 512:(t + 1) * 512])
                ps = pnode.tile([128, 512], f32, name="ps_emb", tag="ps_node")
                nc.tensor.matmul(out=ps[:], lhsT=embW_t[:], rhs=xt[:], start=True, stop=True)
                nc.scalar.activation(out=hT[:, t * 512:(t + 1) * 512], in_=ps[:], func=AF.Copy)
            transpose_to_rows(hT, 0, Npad, 0, bf=True)
            nc.gpsimd.collective_compute(
                "AllGather", OP.bypass, replica_groups=[list(range(M))],
                ins=[h_rows_bf[:].opt()], outs=[h_tables[0][:].opt()])

            stats_small = []

            def small(name):
                t = stp.tile([128, 1], f32, name=name, tag=name)
                return t

            for i in range(L):
                # ================= edge phase =================
                for sb in range(NSB):
                    s_lo = sb * SBW * CPW * 128
                    s_hi = (sb + 1) * SBW * CPW * 128
                    ea = einp.tile([BOND + 1, SBW * CPW * 128], f32, name="ea", tag="ea")
                    nc.sync.dma_start(out=ea[:], in_=P_ea[:, s_lo:s_hi])
                    gat = gatp.tile([128, SBW * CPW * 128], bf16, name="gat", tag="gat")
                    for wl in range(SBW):
                        w = sb * SBW + wl
                        ps_a = pagg.tile([128, 128], f32, name="ps_a", tag="ps_a")
                        for j in range(CPW):
                            ch = wl * CPW + j
                            gcol = w * CPW + j
                            nc.gpsimd.indirect_dma_start(
                                out=gat[:, ch * 128:(ch + 1) * 128], out_offset=None,
                                in_=h_tables[i][:],
                                in_offset=IndirectOffsetOnAxis(ap=gidx_t[:, gcol:gcol + 1], axis=0))
                            ps_e = pse.tile([128, 128], f32, name="ps_e", tag="ps_tr")
                            nc.tensor.matmul(out=ps_e[:], lhsT=ea[:, ch * 128:(ch + 1) * 128],
                                             rhs=bondW_t[:, i * D:(i + 1) * D],
                                             start=True, stop=True)
                            msg = smp.tile([128, 128], f32, name="msg", tag="msg")
                            nc.vector.tensor_tensor(out=msg[:], in0=ps_e[:],
                                                    in1=gat[:, ch * 128:(ch + 1) * 128],
                                                    op=OP.add)
                            nc.scalar.activation(out=msg[:], in_=msg[:], func=AF.Relu)
                            S = smp.tile([128, 128], f32, name="S", tag="S")
                            nc.vector.tensor_tensor(
                                out=S[:], in0=dstin_t[:, gcol:gcol + 1].to_broadcast([128, 128]),
                                in1=iota_t[:], op=OP.is_equal)
                            nc.tensor.matmul(out=ps_a[:], lhsT=msg[:], rhs=S[:],
                                             start=(j == 0), stop=(j == CPW - 1))
                        # z = h + agg (in place)
                        nc.vector.tensor_tensor(out=hT[:, w * 128:(w + 1) * 128],
                                                in0=hT[:, w * 128:(w + 1) * 128],
                                                in1=ps_a[:], op=OP.add)

                # ================= node phase =================
                # Pass A: stats of raw z1 = z @ W1 (no bias; bias shifts mean only,
                # and BN cancels additive bias, so we ignore b1/b2 entirely).
                s1 = [stp.tile([128, NT], f32, name=f"s1{h}", tag=f"s1{h}") for h in range(2)]
                q1 = [stp.tile([128, NT], f32, name=f"q1{h}", tag=f"q1{h}") for h in range(2)]
                padv = [small(f"padv{h}") for h in range(2)]
                for t in range(NT):
                    for h in range(2):
                        ps = pnode.tile([128, 512], f32, name="ps_n", tag="ps_node")
                        nc.tensor.matmul(out=ps[:],
                                         lhsT=W1_t[:, (i * 2 + h) * 128:(i * 2 + h + 1) * 128],
                                         rhs=hT[:, t * 512:(t + 1) * 512], start=True, stop=True)
                        nc.vector.tensor_reduce(out=s1[h][:, t:t + 1], in_=ps[:],
                                                axis=mybir.AxisListType.X, op=OP.add)
                        scr = scrp.tile([128, 512], f32, name="scr", tag="scr")
                        nc.scalar.activation(out=scr[:], in_=ps[:], func=AF.Square,
                                             accum_out=q1[h][:, t:t + 1])
                        if t == NT - 1:
                            nc.vector.tensor_copy(out=padv[h][:], in_=ps[:, 511:512])
                ar_sb = stp.tile([128, 4], f32, name="ar_sb", tag="ar_sb")
                tmp = small("tmp"); tmp2 = small("tmp2")
                for h in range(2):
                    nc.vector.tensor_reduce(out=ar_sb[:, 2 * h:2 * h + 1], in_=s1[h][:],
                                            axis=mybir.AxisListType.X, op=OP.add)
                    nc.vector.tensor_reduce(out=ar_sb[:, 2 * h + 1:2 * h + 2], in_=q1[h][:],
                                            axis=mybir.AxisListType.X, op=OP.add)
                    # subtract padding contribution: kc * v and kc * v^2
                    nc.vector.tensor_tensor(out=tmp[:], in0=padv[h][:], in1=kc_t[:], op=OP.mult)
                    nc.vector.tensor_tensor(out=ar_sb[:, 2 * h:2 * h + 1],
                                            in0=ar_sb[:, 2 * h:2 * h + 1], in1=tmp[:], op=OP.subtract)
                    nc.vector.tensor_tensor(out=tmp2[:], in0=padv[h][:], in1=padv[h][:], op=OP.mult)
                    nc.vector.tensor_tensor(out=tmp2[:], in0=tmp2[:], in1=kc_t[:], op=OP.mult)
                    nc.vector.tensor_tensor(out=ar_sb[:, 2 * h + 1:2 * h + 2],
                                            in0=ar_sb[:, 2 * h + 1:2 * h + 2], in1=tmp2[:], op=OP.subtract)
                nc.sync.dma_start(out=ar_ins[2 * i][:], in_=ar_sb[:])
                nc.gpsimd.collective_compute(
                    "AllReduce", OP.add, replica_groups=[list(range(M))],
                    ins=[ar_ins[2 * i][:].opt()], outs=[ar_outs[2 * i][:].opt()])
                st1 = stp.tile([128, 4], f32, name="st1", tag="st1")
                nc.sync.dma_start(out=st1[:], in_=ar_outs[2 * i][:])

                A1 = [small(f"A1{h}") for h in range(2)]
                B1 = [small(f"B1{h}") for h in range(2)]
                mny = small("mny"); mq = small("mq"); var = small("var"); sd = small("sd")
                for h in range(2):
                    nc.vector.tensor_scalar_mul(out=mny[:], in0=st1[:, 2 * h:2 * h + 1], scalar1=invN)
                    nc.vector.tensor_scalar_mul(out=mq[:], in0=st1[:, 2 * h + 1:2 * h + 2], scalar1=invN)
                    nc.vector.tensor_tensor(out=var[:], in0=mny[:], in1=mny[:], op=OP.mult)
                    nc.vector.tensor_tensor(out=var[:], in0=mq[:], in1=var[:], op=OP.subtract)
                    nc.scalar.activation(out=sd[:], in_=var[:], func=AF.Sqrt, bias=eps_t[:, :1])
                    nc.vector.reciprocal(out=A1[h][:], in_=sd[:])
                    nc.vector.tensor_tensor(out=A1[h][:], in0=A1[h][:],
                                            in1=g1_t[:, i * 2 + h:i * 2 + h + 1], op=OP.mult)
                    nc.vector.tensor_tensor(out=B1[h][:], in0=mny[:], in1=A1[h][:], op=OP.mult)
                    nc.vector.tensor_tensor(out=B1[h][:], in0=be1_t[:, i * 2 + h:i * 2 + h + 1],
                                            in1=B1[h][:], op=OP.subtract)

                # Pass B: recompute z1, apply BN1+relu, z2 = z1n @ W2, stats of h_pre
                s2 = stp.tile([128, NT], f32, name="s2", tag="s2")
                q2 = stp.tile([128, NT], f32, name="q2", tag="q2")
                padv2 = small("padv2")
                for t in range(NT):
                    ps2 = pacc.tile([128, 512], f32, name="ps2", tag="ps2")
                    for h in range(2):
                        ps = pnode.tile([128, 512], f32, name="ps_n", tag="ps_node")
                        nc.tensor.matmul(out=ps[:],
                                         lhsT=W1_t[:, (i * 2 + h) * 128:(i * 2 + h + 1) * 128],
                                         rhs=hT[:, t * 512:(t + 1) * 512], start=True, stop=True)
                        z1n = z1np.tile([128, 512], f32, name="z1n", tag="z1n")
                        nc.scalar.activation(out=z1n[:], in_=ps[:], func=AF.Relu,
                                             scale=A1[h][:, :1], bias=B1[h][:, :1])
                        nc.tensor.matmul(out=ps2[:],
                                         lhsT=W2_t[:, (i * 2 + h) * 128:(i * 2 + h + 1) * 128],
                                         rhs=z1n[:], start=(h == 0), stop=(h == 1))
                    nc.scalar.activation(out=h2T[:, t * 512:(t + 1) * 512], in_=ps2[:],
                                         func=AF.Copy, accum_out=s2[:, t:t + 1])
                    scr = scrp.tile([128, 512], f32, name="scr", tag="scr")
                    nc.scalar.activation(out=scr[:], in_=h2T[:, t * 512:(t + 1) * 512],
                                         func=AF.Square, accum_out=q2[:, t:t + 1])
                    if t == NT - 1:
                        nc.vector.tensor_copy(out=padv2[:], in_=ps2[:, 511:512])
                ar_sb2 = stp.tile([128, 4], f32, name="ar_sb2", tag="ar_sb")
                nc.vector.memset(ar_sb2[:, 2:4], 0.0)
                nc.vector.tensor_reduce(out=ar_sb2[:, 0:1], in_=s2[:],
                                        axis=mybir.AxisListType.X, op=OP.add)
                nc.vector.tensor_reduce(out=ar_sb2[:, 1:2], in_=q2[:],
                                        axis=mybir.AxisListType.X, op=OP.add)
                nc.vector.tensor_tensor(out=tmp[:], in0=padv2[:], in1=kc_t[:], op=OP.mult)
                nc.vector.tensor_tensor(out=ar_sb2[:, 0:1], in0=ar_sb2[:, 0:1], in1=tmp[:], op=OP.subtract)
                nc.vector.tensor_tensor(out=tmp2[:], in0=padv2[:], in1=padv2[:], op=OP.mult)
                nc.vector.tensor_tensor(out=tmp2[:], in0=tmp2[:], in1=kc_t[:], op=OP.mult)
                nc.vector.tensor_tensor(out=ar_sb2[:, 1:2], in0=ar_sb2[:, 1:2], in1=tmp2[:], op=OP.subtract)
                nc.sync.dma_start(out=ar_ins[2 * i + 1][:], in_=ar_sb2[:])
                nc.gpsimd.collective_compute(
                    "AllReduce", OP.add, replica_groups=[list(range(M))],
                    ins=[ar_ins[2 * i + 1][:].opt()], outs=[ar_outs[2 * i + 1][:].opt()])
                st2 = stp.tile([128, 4], f32, name="st2", tag="st1")
                nc.sync.dma_start(out=st2[:], in_=ar_outs[2 * i + 1][:])

                A2 = small("A2"); B2 = small("B2")
                nc.vector.tensor_scalar_mul(out=mny[:], in0=st2[:, 0:1], scalar1=invN)
                nc.vector.tensor_scalar_mul(out=mq[:], in0=st2[:, 1:2], scalar1=invN)
                nc.vector.tensor_tensor(out=var[:], in0=mny[:], in1=mny[:], op=OP.mult)
                nc.vector.tensor_tensor(out=var[:], in0=mq[:], in1=var[:], op=OP.subtract)
                nc.scalar.activation(out=sd[:], in_=var[:], func=AF.Sqrt, bias=eps_t[:, :1])
                nc.vector.reciprocal(out=A2[:], in_=sd[:])
                nc.vector.tensor_tensor(out=A2[:], in0=A2[:], in1=gout_t[:, i:i + 1], op=OP.mult)
                nc.vector.tensor_tensor(out=B2[:], in0=mny[:], in1=A2[:], op=OP.mult)
                nc.vector.tensor_tensor(out=B2[:], in0=bout_t[:, i:i + 1], in1=B2[:], op=OP.subtract)

                # Pass C: h = act(h_pre * A2 + B2); rows + AllGather for next layer
                fn = AF.Relu if i < L - 1 else AF.Identity
                for t in range(NT):
                    nc.scalar.activation(out=hT[:, t * 512:(t + 1) * 512],
                                         in_=h2T[:, t * 512:(t + 1) * 512],
                                         func=fn, scale=A2[:, :1], bias=B2[:, :1])
                if i < L - 1:
                    transpose_to_rows(hT, 0, Npad, 0, bf=True)
                    nc.gpsimd.collective_compute(
                        "AllGather", OP.bypass, replica_groups=[list(range(M))],
                        ins=[h_rows_bf[:].opt()], outs=[h_tables[i + 1][:].opt()])

            # ================= pooling =================
            # exclusive prefix sums along nodes, stored as rows; pooled mean by
            # two indirect row gathers (lo/hi) per 128-graph window.
            pref = h2T
            carry = stp.tile([128, 1], f32, name="carry", tag="carry")
            nc.vector.memset(carry[:], 0.0)
            for t in range(NT):
                nc.vector.tensor_tensor_scan(
                    out=pref[:, t * 512:(t + 1) * 512],
                    data0=hT[:, t * 512:(t + 1) * 512],
                    data1=hT[:, t * 512:(t + 1) * 512],
                    initial=carry[:, :1] if t > 0 else 0.0,
                    op0=OP.add, op1=OP.bypass)
                if t < NT - 1:
                    nc.vector.tensor_copy(out=carry[:], in_=pref[:, (t + 1) * 512 - 1:(t + 1) * 512])
            # write exclusive-prefix rows: h_rows[0] = 0, h_rows[r] = pref[:, r-1]
            zrow = stp.tile([128, 1], f32, name="zrow", tag="zrow")
            nc.vector.memset(zrow[:], 0.0)
            zsb = smp.tile([128, 128], f32, name="zsb", tag="rsb")
            nc.vector.memset(zsb[:], 0.0)
            nc.sync.dma_start(out=h_rows[0:1, :], in_=zsb[:1, :])
            transpose_to_rows(pref, 0, Npad, 1)
            from concourse.bass import IndirectOffsetOnAxis as IOA
            for pw in range(NPW):
                glo = smp.tile([128, D], f32, name="glo", tag="glo")
                ghi = smp.tile([128, D], f32, name="ghi", tag="ghi")
                nc.gpsimd.indirect_dma_start(out=glo[:], out_offset=None, in_=h_rows[:],
                                             in_offset=IOA(ap=plo_t[:, pw:pw + 1], axis=0))
                nc.gpsimd.indirect_dma_start(out=ghi[:], out_offset=None, in_=h_rows[:],
                                             in_offset=IOA(ap=phi_t[:, pw:pw + 1], axis=0))
                osb = smp.tile([128, D], f32, name="osb", tag="osb")
                nc.vector.tensor_tensor(out=osb[:], in0=ghi[:], in1=glo[:], op=OP.subtract)
                nc.vector.tensor_scalar_mul(out=osb[:], in0=osb[:], scalar1=pscale_t[:, pw:pw + 1])
                nc.sync.dma_start(out=P_out[pw * 128:(pw + 1) * 128, :], in_=osb[:])

    nc.finalize()
    return nc


def kernel(_G=G_DEFAULT, _trace=False, **inputs):
    _register_ntff_hook()
    from concourse.bass_utils import run_bass_kernel_spmd

    cfg, shared, per_core = preprocess(G=_G, **inputs)
    nc = build(cfg)

    in_maps = []
    for c in range(M):
        m = dict(shared)
        m["xT"] = per_core["xT"][c]
        m["eattrT"] = per_core["eattrT"][c]
        m["gidx"] = per_core["gidx"][c]
        m["dstin"] = per_core["dstin"][c]
        m["kc"] = per_core["kc"][c]
        m["plo"] = per_core["plo"][c]
        m["phi"] = per_core["phi"][c]
        m["pscale"] = per_core["pscale"][c]
        in_maps.append(m)

    res = run_bass_kernel_spmd(nc, in_maps, list(range(M)), trace=_trace)

    G = cfg["G"]; D = cfg["D"]
    out = np.zeros((G, D), np.float32)
    gs = cfg["graph_start"]
    for c in range(M):
        ng = int(cfg["g_real"][c])
        out[gs[c]:gs[c] + ng] = res.results[c]["out"][:ng]
    kernel._last_exec_ns = res.exec_time_ns
    return out
